# revision 16
# baseline (speedup 1.0000x reference)
"""Trainium2 Bass kernel for a 3x3 stride-1 pad-1 Conv2d.

Problem: x (16, 64, 112, 112) f32, weights (128, 64, 9) f32
         -> out (16, 128, 112, 112) f32  (no bias)

Strategy (8 NeuronCores, data parallel over batch):
  - Each core gets 2 images. Image 0 lives in SBUF partitions 0-63
    (64 input channels), image 1 in partitions 64-127, both stored as a
    zero-padded (114, 114) plane per channel.
  - Conv = 9 shift-and-matmul taps accumulated in PSUM: for each tap
    (dy, dx), matmul with lhsT = w[tap] (64 x M: in-ch x out-ch) and
    rhs = shifted x window (64 x 448: in-ch x 4 output rows).
  - The two images' matmuls use disjoint PE row groups (rows 0-63 vs
    64-127 via tile_position), so they execute concurrently. With
    COLSPLIT, each tap is further split into two M=64 column groups so
    the fp32r self-weight-load (which serializes with matmuls inside
    one array region) halves.
  - float32r matmuls: full-rate (1 cycle/row) fp32 path at free-dim
    >= 256.
  - Inputs land in SBUF via contiguous-descriptor DMAs into a staging
    tile, then VectorE spreads them into the padded plane (avoids the
    448-byte-descriptor DMA tax of strided writes). Outputs are staged
    per 16-row band and stored with large descriptors. PSUM -> SBUF on
    ScalarE (same engine issues the store DMA, so no extra sem wait).
"""

import numpy as np

import concourse.bass as bass
import concourse.bacc as bacc
import concourse.mybir as mybir
import concourse.tile as tile
from concourse.bass_utils import run_bass_kernel_spmd

N_CORES = 8
B, C, H, W = 16, 64, 112, 112
O = 128
BPC = B // N_CORES          # images per core
HP = H + 2                  # padded rows per image plane
WP = W + 2                  # padded cols
NTAPS = 9
RPB = 4                     # output rows per block (free dim = 4*112 = 448)
NBLOCKS = H // RPB          # 28
BAND = 16                   # output rows per input/output band
NBANDS = H // BAND          # 7

COLSPLIT = False            # split M=128 into two concurrent M=64 col groups

F32 = mybir.dt.float32
F32R = mybir.dt.float32r
U32 = mybir.dt.uint32


# input bands: (dst padded row, src input row, nrows); the first band is
# split into a small head so the PE can start after ~0.5 MB of input.
def _in_bands():
    bands = [(1, 0, 6), (7, 6, BAND + 3 - 6)]       # rows 0..5, 6..18
    for b in range(1, NBANDS):
        r0 = BAND * b + 3
        n = min(BAND, H - r0)                       # last band: 13 rows
        bands.append((r0 + 1, r0, n))
    return bands


_BANDS = _in_bands()


def _max_rows():
    return max(n for _, _, n in _BANDS)


def _conv_body(tc, out_ap, x_ap, w_ap):
    nc = tc.nc
    from contextlib import ExitStack

    with ExitStack() as ctx:
        xpool = ctx.enter_context(tc.tile_pool(name="xb", bufs=1))
        wpool = ctx.enter_context(tc.tile_pool(name="wt", bufs=1))
        inpool = ctx.enter_context(tc.tile_pool(name="ins", bufs=2))
        pspool = ctx.enter_context(tc.tile_pool(name="ps", bufs=3, space="PSUM"))
        opool = ctx.enter_context(tc.tile_pool(name="ob", bufs=2))

        # x planes: partitions [64*im, 64*im+64) hold image im, padded.
        xb = xpool.tile([128, HP, WP], F32R)
        # weights: wt[p, t, m] = w[m, p % 64, t] (taps replicated per half)
        wt = wpool.tile([128, NTAPS, O], F32R)

        # head band of x first: these two small DMAs gate the first matmul
        dst_r0, src_r0, n0 = _BANDS[0]
        stg0 = inpool.tile([128, _max_rows(), W], F32R, name="stg_head", tag="stg")
        for im in range(BPC):
            p0 = 64 * im
            nc.sync.dma_start(
                out=stg0[p0:p0 + 64, 0:n0, :],
                in_=x_ap[im, :, src_r0:src_r0 + n0, :],
            )

        nc.sync.dma_start(out=wt[:], in_=w_ap[:])

        # zero borders (cols 0/113, rows 0/113); spreads fill the interior
        nc.vector.memset(xb[:, :, 0].bitcast(U32), 0)
        nc.vector.memset(xb[:, :, WP - 1].bitcast(U32), 0)
        nc.vector.memset(xb[:, 0, :].bitcast(U32), 0)
        nc.vector.memset(xb[:, HP - 1, :].bitcast(U32), 0)

        # PE warmup: dummy matmuls on the weights keep the HAM activity
        # window open while the first x band is still in flight, so the
        # real matmul stream starts at 2.4 GHz.
        scr = pspool.tile([128, RPB, W], F32, tag="scr", name="scr", bufs=1)
        for wmup in range(12):
            nc.tensor.matmul(
                scr[:, 0:RPB, 0:112],
                wt[0:64, 0, :],
                wt[0:64, 0:RPB, 0:112],
                start=True,
                stop=True,
                tile_position=(0, 0),
                skip_group_check=True,
            )

        # banded loads: HBM -> contiguous staging (fat descriptors), then
        # VectorE spreads into the padded plane (both images at once).
        nc.vector.tensor_copy(
            xb[:, dst_r0:dst_r0 + n0, 1:1 + W],
            stg0[:, 0:n0, :],
        )
        for b, (dst_r, src_r, n) in enumerate(_BANDS[1:], start=1):
            stg = inpool.tile([128, _max_rows(), W], F32R, name=f"stg{b}", tag="stg")
            for im in range(BPC):
                p0 = 64 * im
                nc.sync.dma_start(
                    out=stg[p0:p0 + 64, 0:n, :],
                    in_=x_ap[im, :, src_r:src_r + n, :],
                )
            nc.vector.tensor_copy(
                xb[:, dst_r:dst_r + n, 1:1 + W],
                stg[:, 0:n, :],
            )

        if COLSPLIT:
            units = [(0, 0), (0, 64), (64, 0), (64, 64)]  # (row base, col base)
        else:
            units = [(0, 0), (64, 0)]
        mwid = 64 if COLSPLIT else 128

        ob_tiles = {}
        for p in range(NBLOCKS):
            r = RPB * p
            band = r // BAND
            boff = r - band * BAND
            if boff == 0:
                for im in range(BPC):
                    ob_tiles[im] = opool.tile(
                        [128, BAND, W], F32, name=f"ob{im}_{band}", tag=f"ob{im}"
                    )
            ps = [
                pspool.tile([128, RPB, W], F32, tag=f"ps{im}", name=f"ps{im}_{p}")
                for im in range(BPC)
            ]
            for t in range(NTAPS):
                i, j = divmod(t, 3)
                first, last = t == 0, t == NTAPS - 1
                for (p0, m0) in units:
                    im = p0 // 64
                    nc.tensor.matmul(
                        ps[im][m0:m0 + mwid, :, :],
                        wt[p0:p0 + 64, t, m0:m0 + mwid],
                        xb[p0:p0 + 64, r + i:r + i + RPB, j:j + W],
                        start=first,
                        stop=last,
                        tile_position=(p0, m0),
                        skip_group_check=COLSPLIT,
                    )
            for im in range(BPC):
                nc.scalar.copy(ob_tiles[im][:, boff:boff + RPB, :], ps[im][:])
            last_band = band == NBANDS - 1
            if last_band:
                for im in range(BPC):
                    nc.scalar.dma_start(
                        out=out_ap[im, :, r:r + RPB, :],
                        in_=ob_tiles[im][:, boff:boff + RPB, :],
                    )
            elif boff + RPB == BAND:
                for im in range(BPC):
                    nc.scalar.dma_start(
                        out=out_ap[im, :, band * BAND:(band + 1) * BAND, :],
                        in_=ob_tiles[im][:],
                    )


def build_program():
    nc = bacc.Bacc("TRN2", target_bir_lowering=False, num_devices=N_CORES)
    x_t = nc.dram_tensor("x", [BPC, C, H, W], F32R, kind="ExternalInput")
    w_t = nc.dram_tensor("wT", [128, NTAPS, O], F32R, kind="ExternalInput")
    o_t = nc.dram_tensor("out", [BPC, O, H, W], F32, kind="ExternalOutput")
    with tile.TileContext(nc) as tc:
        _conv_body(tc, o_t.ap(), x_t.ap(), w_t.ap())
    nc.compile()
    return nc


def pack_weights(weights: np.ndarray) -> np.ndarray:
    # (O, C, 9) -> (128, 9, O) with wT[p, t, m] = weights[m, p % 64, t]
    wT = np.ascontiguousarray(np.transpose(weights, (1, 2, 0)))  # (C, 9, O)
    return np.ascontiguousarray(np.concatenate([wT, wT], axis=0))


def run(x: np.ndarray, weights: np.ndarray, **spmd_kwargs):
    x = np.ascontiguousarray(x, dtype=np.float32)
    w = np.ascontiguousarray(weights, dtype=np.float32)
    wT = pack_weights(w)
    nc = build_program()
    in_maps = [{"x": x[BPC * i:BPC * (i + 1)], "wT": wT} for i in range(N_CORES)]
    res = run_bass_kernel_spmd(nc, in_maps, list(range(N_CORES)), **spmd_kwargs)
    outs = [
        np.asarray(res.results[i]["out"]).reshape(BPC, O, H, W)
        for i in range(N_CORES)
    ]
    return np.concatenate(outs, axis=0), res


def kernel(x: np.ndarray, weights: np.ndarray) -> np.ndarray:
    out, _ = run(x, weights)
    return out


# revision 17
# speedup vs baseline: 1.1162x; 1.1162x over previous
"""Trainium2 Bass kernel for a 3x3 stride-1 pad-1 Conv2d.

Problem: x (16, 64, 112, 112) f32, weights (128, 64, 9) f32
         -> out (16, 128, 112, 112) f32  (no bias)

Strategy (8 NeuronCores, data parallel over batch):
  - Each core gets 2 images. Image 0 lives in SBUF partitions 0-63
    (64 input channels), image 1 in partitions 64-127, both stored as a
    zero-padded (114, 114) plane per channel.
  - Conv = 9 shift-and-matmul taps accumulated in PSUM: for each tap
    (dy, dx), matmul with lhsT = w[tap] (64 x M: in-ch x out-ch) and
    rhs = shifted x window (64 x 448: in-ch x 4 output rows).
  - The two images' matmuls use disjoint PE row groups (rows 0-63 vs
    64-127 via tile_position), so they execute concurrently. With
    COLSPLIT, each tap is further split into two M=64 column groups so
    the fp32r self-weight-load (which serializes with matmuls inside
    one array region) halves.
  - float32r matmuls: full-rate (1 cycle/row) fp32 path at free-dim
    >= 256.
  - Inputs land in SBUF via contiguous-descriptor DMAs into a staging
    tile, then VectorE spreads them into the padded plane (avoids the
    448-byte-descriptor DMA tax of strided writes). Outputs are staged
    per 16-row band and stored with large descriptors. PSUM -> SBUF on
    ScalarE (same engine issues the store DMA, so no extra sem wait).
"""

import numpy as np

import concourse.bass as bass
import concourse.bacc as bacc
import concourse.mybir as mybir
import concourse.tile as tile
from concourse.bass_utils import run_bass_kernel_spmd

N_CORES = 8
B, C, H, W = 16, 64, 112, 112
O = 128
BPC = B // N_CORES          # images per core
HP = H + 2                  # padded rows per image plane
WP = W + 2                  # padded cols
NTAPS = 9
RPB = 4                     # output rows per block (free dim = 4*112 = 448)
NBLOCKS = H // RPB          # 28
BAND = 16                   # output rows per input/output band
NBANDS = H // BAND          # 7

COLSPLIT = False            # split M=128 into two concurrent M=64 col groups

F32 = mybir.dt.float32
F32R = mybir.dt.float32r
U32 = mybir.dt.uint32


# input bands: (dst padded row, src input row, nrows); the first band is
# split into a small head so the PE can start after ~0.5 MB of input.
def _in_bands():
    bands = [(1, 0, 6), (7, 6, BAND + 3 - 6)]       # rows 0..5, 6..18
    for b in range(1, NBANDS):
        r0 = BAND * b + 3
        n = min(BAND, H - r0)                       # last band: 13 rows
        bands.append((r0 + 1, r0, n))
    return bands


_BANDS = _in_bands()


def _max_rows():
    return max(n for _, _, n in _BANDS)


def _conv_body(tc, out_ap, x_ap, w_ap):
    nc = tc.nc
    from contextlib import ExitStack

    with ExitStack() as ctx:
        xpool = ctx.enter_context(tc.tile_pool(name="xb", bufs=1))
        wpool = ctx.enter_context(tc.tile_pool(name="wt", bufs=1))
        inpool = ctx.enter_context(tc.tile_pool(name="ins", bufs=2))
        pspool = ctx.enter_context(tc.tile_pool(name="ps", bufs=4, space="PSUM"))
        opool = ctx.enter_context(tc.tile_pool(name="ob", bufs=2))

        # x planes: partitions [64*im, 64*im+64) hold image im, padded.
        xb = xpool.tile([128, HP, WP], F32R)
        # weights: wt[p, t, m] = w[m, p % 64, t] (taps replicated per half)
        wt = wpool.tile([128, NTAPS, O], F32R)

        # weights first (gates the first matmul), then the small head band
        nc.sync.dma_start(out=wt[:], in_=w_ap[:])

        dst_r0, src_r0, n0 = _BANDS[0]
        stg0 = inpool.tile([128, _max_rows(), W], F32R, name="stg_head", tag="stg")
        for im in range(BPC):
            p0 = 64 * im
            nc.sync.dma_start(
                out=stg0[p0:p0 + 64, 0:n0, :],
                in_=x_ap[im, :, src_r0:src_r0 + n0, :],
            )

        # zero borders (cols 0/113, rows 0/113); spreads fill the interior
        nc.vector.memset(xb[:, :, 0].bitcast(U32), 0)
        nc.vector.memset(xb[:, :, WP - 1].bitcast(U32), 0)
        nc.vector.memset(xb[:, 0, :].bitcast(U32), 0)
        nc.vector.memset(xb[:, HP - 1, :].bitcast(U32), 0)

        # banded loads: HBM -> contiguous staging (fat descriptors), then
        # VectorE spreads into the padded plane (both images at once).
        nc.vector.tensor_copy(
            xb[:, dst_r0:dst_r0 + n0, 1:1 + W],
            stg0[:, 0:n0, :],
        )
        for b, (dst_r, src_r, n) in enumerate(_BANDS[1:], start=1):
            stg = inpool.tile([128, _max_rows(), W], F32R, name=f"stg{b}", tag="stg")
            for im in range(BPC):
                p0 = 64 * im
                nc.sync.dma_start(
                    out=stg[p0:p0 + 64, 0:n, :],
                    in_=x_ap[im, :, src_r:src_r + n, :],
                )
            nc.vector.tensor_copy(
                xb[:, dst_r:dst_r + n, 1:1 + W],
                stg[:, 0:n, :],
            )

        if COLSPLIT:
            units = [(0, 0), (0, 64), (64, 0), (64, 64)]  # (row base, col base)
        else:
            units = [(0, 0), (64, 0)]
        mwid = 64 if COLSPLIT else 128

        ob_tiles = {}
        for p in range(NBLOCKS):
            r = RPB * p
            band = r // BAND
            boff = r - band * BAND
            if boff == 0:
                for im in range(BPC):
                    ob_tiles[im] = opool.tile(
                        [128, BAND, W], F32, name=f"ob{im}_{band}", tag=f"ob{im}"
                    )
            ps = [
                pspool.tile([128, RPB, W], F32, tag=f"ps{im}", name=f"ps{im}_{p}")
                for im in range(BPC)
            ]
            for t in range(NTAPS):
                i, j = divmod(t, 3)
                first, last = t == 0, t == NTAPS - 1
                for (p0, m0) in units:
                    im = p0 // 64
                    nc.tensor.matmul(
                        ps[im][m0:m0 + mwid, :, :],
                        wt[p0:p0 + 64, t, m0:m0 + mwid],
                        xb[p0:p0 + 64, r + i:r + i + RPB, j:j + W],
                        start=first,
                        stop=last,
                        tile_position=(p0, m0),
                        skip_group_check=COLSPLIT,
                    )
            for im in range(BPC):
                nc.scalar.copy(ob_tiles[im][:, boff:boff + RPB, :], ps[im][:])
            last_band = band == NBANDS - 1
            if last_band:
                for im in range(BPC):
                    nc.scalar.dma_start(
                        out=out_ap[im, :, r:r + RPB, :],
                        in_=ob_tiles[im][:, boff:boff + RPB, :],
                    )
            elif boff + RPB == BAND:
                for im in range(BPC):
                    nc.scalar.dma_start(
                        out=out_ap[im, :, band * BAND:(band + 1) * BAND, :],
                        in_=ob_tiles[im][:],
                    )


def build_program():
    nc = bacc.Bacc("TRN2", target_bir_lowering=False, num_devices=N_CORES)
    x_t = nc.dram_tensor("x", [BPC, C, H, W], F32R, kind="ExternalInput")
    w_t = nc.dram_tensor("wT", [128, NTAPS, O], F32R, kind="ExternalInput")
    o_t = nc.dram_tensor("out", [BPC, O, H, W], F32, kind="ExternalOutput")
    with tile.TileContext(nc) as tc:
        _conv_body(tc, o_t.ap(), x_t.ap(), w_t.ap())
    nc.compile()
    return nc


def pack_weights(weights: np.ndarray) -> np.ndarray:
    # (O, C, 9) -> (128, 9, O) with wT[p, t, m] = weights[m, p % 64, t]
    wT = np.ascontiguousarray(np.transpose(weights, (1, 2, 0)))  # (C, 9, O)
    return np.ascontiguousarray(np.concatenate([wT, wT], axis=0))


def run(x: np.ndarray, weights: np.ndarray, **spmd_kwargs):
    x = np.ascontiguousarray(x, dtype=np.float32)
    w = np.ascontiguousarray(weights, dtype=np.float32)
    wT = pack_weights(w)
    nc = build_program()
    in_maps = [{"x": x[BPC * i:BPC * (i + 1)], "wT": wT} for i in range(N_CORES)]
    res = run_bass_kernel_spmd(nc, in_maps, list(range(N_CORES)), **spmd_kwargs)
    outs = [
        np.asarray(res.results[i]["out"]).reshape(BPC, O, H, W)
        for i in range(N_CORES)
    ]
    return np.concatenate(outs, axis=0), res


def kernel(x: np.ndarray, weights: np.ndarray) -> np.ndarray:
    out, _ = run(x, weights)
    return out
